# revision 41
# baseline (speedup 1.0000x reference)
# DeepSeek block (MLA attention + top-2-of-8 MoE + shared expert) on 8 TRN2
# NeuronCores, zero-collective sharding.
#
# Core c in [0..8): sequence b = c//4, q = c%4; owns token chunks
# hi = 7-q (slot 0) and lo = q (slot 1), 256 tokens each (causally balanced:
# every core's true causal work is 9 key blocks of 256).
#
# The SPMD program is identical on all cores; everything core-specific
# arrives as data: slot 0 attends key blocks [0..16), slot 1 [0..8), with
# host-built multiplicative masks (ones for fully-past blocks, triangular at
# the causal boundary, zeros for fully-future blocks).
#
# Layout: activations live feature-on-partition ("^T", tokens on the free
# axis). Host pre-transposes x, folds w_ln1/w_ln2 into adjacent weights,
# pre-casts weights to bf16, and builds rope tables / masks / identity.
# Matmuls run in bf16 (fp32 PSUM accumulate); the gate runs in fp32.
#
# Engine-balance notes (cost model): Activation has ~370ns fixed cost per
# instruction, so softmax exps are batched 512 cols wide (two key blocks per
# exp). DMA generation costs ~625ns/instruction, so kfull is assembled with
# 4 DMAs per tile and the gating weights reach all partitions via gpsimd
# row-broadcasts instead of DMA scatters.
import os
import numpy as np
import ml_dtypes

import concourse.bacc as bacc
import concourse.mybir as mybir
import concourse.tile as tile
from concourse import bass_utils

F32 = mybir.dt.float32
F32R = mybir.dt.float32r
BF16 = mybir.dt.bfloat16
F8 = mybir.dt.float8e4
DR = mybir.MatmulPerfMode.DoubleRow
AF = mybir.ActivationFunctionType
ALU = mybir.AluOpType
WS = 64.0                # fp8 weight pre-scale (2^6); undone at the flush

B, T, C, H, D = 2, 2048, 1024, 16, 64
R, ROPE, NOPE = 128, 32, 32
E, I = 8, 512
THETA, EPS = 100000.0, 1e-5
P = 128
NCB = C // P             # 8 C blocks
NTB = T // P             # 16 key/token blocks per sequence
TLOC, CHUNK = 512, 256
QB = 128                 # query slot width
KB_SLOT = (16, 12, 8, 4)  # key blocks (of 128) attended per query slot
NKB = sum(KB_SLOT)       # 40
NIB = I // P             # 4 I blocks

DEBUG = bool(int(os.environ.get("BASSK_DEBUG", "0")))
_CACHE = {}


# =============================================================== device IR
def _emit(nc, tc):
    import contextlib

    def din(name, shape, dt):
        return nc.dram_tensor(name, shape, dt, kind="ExternalInput")

    xT_f8  = din("xT_f8", (C, T), F8)
    xlocT  = din("xlocT", (C, TLOC), F32)
    xlocbf = din("xlocbf", (C, TLOC), BF16)
    wq     = din("wq", (P, NCB, NCB, P), BF16)
    wkva   = din("wkva", (C, R + ROPE), F8)
    wkvb   = din("wkvb", (R, H * NOPE), BF16)
    wo     = din("wo", (H * NOPE, C), BF16)
    cosq   = din("cosq", (P, TLOC), BF16)
    ssinq  = din("ssinq", (P, TLOC), BF16)
    cosk   = din("cosk", (ROPE, T), BF16)
    ssink  = din("ssink", (ROPE, T), BF16)
    perm64 = din("perm64", (2 * ROPE, 2 * ROPE), BF16)
    perm32 = din("perm32", (ROPE, ROPE), BF16)
    ident  = din("ident", (P, P), F32)
    kmask  = din("kmask", (P, 16 * QB), BF16)
    wgate  = din("wgate", (C, E), F32)
    biasg  = din("biasg", (P, E), F32)
    # fp8 expert weights, pre-scaled by 2^6 and packed in DoubleRow k-tile
    # layout: gate/up (128, 4 cb-pairs, 2, I); down (128, 2 ib-pairs, 2, C).
    # Index 0 is the shared expert.
    w8g = din("w8g", (E + 1, P, NCB // 2, 2, I), F8)
    w8u = din("w8u", (E + 1, P, NCB // 2, 2, I), F8)
    w8d = din("w8d", (E + 1, P, NIB // 2, 2, C), F8)

    outT = nc.dram_tensor("outT", (C, TLOC), F32, kind="ExternalOutput")
    dbg = {}
    if DEBUG:
        for name, shape, dt in [
                ("d_xaT", (C, TLOC), F32), ("d_comb", (P, 4 * E), BF16),
                ("d_invr1", (1, T), F32), ("d_invr2", (1, TLOC), F32),
                ("d_xmoe", (C, TLOC), F32), ("d_h0", (P, NIB * CHUNK), BF16),
                ("d_acc0", (C, CHUNK), BF16)]:
            dbg[name] = nc.dram_tensor(name, shape, dt, kind="ExternalOutput")

    # ---------------- pools (sized to fit 192KB/partition SBUF, 8 PSUM banks)
    # K-side tensors (knope/vext/kfull) are emitted BEFORE the Q projection
    # so the Activation-bound score phase can start as soon as qbf[0] lands.
    whole = contextlib.ExitStack()   # whole kernel
    attn  = contextlib.ExitStack()   # until gate done
    early = contextlib.ExitStack()   # until Q/ckv/krope done
    xload = contextlib.ExitStack()   # xt tiles, freed after ckv
    pc   = whole.enter_context(tc.tile_pool(name="pc", bufs=1))
    pmx  = whole.enter_context(tc.tile_pool(name="pmx", bufs=1, side="right"))
    pps  = attn.enter_context(tc.tile_pool(name="psA", bufs=2, space="PSUM"))
    pacc = attn.enter_context(tc.tile_pool(name="psB", bufs=2, space="PSUM"))
    px   = early.enter_context(tc.tile_pool(name="px", bufs=1))
    ptmp1 = early.enter_context(tc.tile_pool(name="tmp1", bufs=2))
    pq_t = early.enter_context(tc.tile_pool(name="pq_t", bufs=2))
    kvst = contextlib.ExitStack()    # kvlat/kropebf/wkvb, freed post-kfull
    pkv1 = kvst.enter_context(tc.tile_pool(name="pkv1", bufs=1))
    pkxst = contextlib.ExitStack()   # knopea, freed after kfull assembly
    pkx = pkxst.enter_context(tc.tile_pool(name="pkx", bufs=1))
    tabs = contextlib.ExitStack()    # K rope tables, freed after rope-K
    ptab = tabs.enter_context(tc.tile_pool(name="ptab", bufs=1))
    ptabt = tabs.enter_context(tc.tile_pool(name="ptabt", bufs=2))
    pxt  = xload.enter_context(tc.tile_pool(name="pxt", bufs=1))

    # ---- bulk loads first, spread over three DGE queues so issue overhead
    # (~625ns/DMA per queue) doesn't serialize the ramp: SP carries the
    # chunk-stat critical path (xt), Act the K-side weights, DVE the Q path.
    xt = []
    for cb in range(NCB):
        tl = pxt.tile([P, T], F8, name=f"xt{cb}")
        nc.sync.dma_start(tl[:], xT_f8.ap()[cb * P:(cb + 1) * P, :])
        xt.append(tl)
    wkva_sb = []
    for cb in range(NCB):
        tl = pxt.tile([P, R + ROPE], F8, name=f"wkvas{cb}")
        nc.scalar.dma_start(tl[:], wkva.ap()[cb * P:(cb + 1) * P, :])
        wkva_sb.append(tl)
    wkvb_sb = pkv1.tile([R, H * NOPE], BF16, name="wkvbs")
    nc.scalar.dma_start(wkvb_sb[:], wkvb.ap())
    xlbf = []
    for cb in range(NCB):
        tb_ = px.tile([P, TLOC], BF16, name=f"xlbf{cb}")
        nc.gpsimd.dma_start(tb_[:], xlocbf.ap()[cb * P:(cb + 1) * P, :])
        xlbf.append(tb_)

    # ---- constants
    ones_128x1 = pc.tile([P, 1], BF16); nc.any.memset(ones_128x1[:], 1.0)
    ones1f = pc.tile([1, 1], F32); nc.any.memset(ones1f[:], 1.0)
    eps_sb = pc.tile([1, 1], F32); nc.any.memset(eps_sb[:], EPS)
    ktab = ptab.tile([2 * ROPE, T], BF16)
    nc.sync.dma_start(ktab[0:ROPE, :], cosk.ap())
    nc.sync.dma_start(ktab[ROPE:2 * ROPE, :], ssink.ap())
    perm64_sb = pc.tile([2 * ROPE, 2 * ROPE], BF16)
    nc.scalar.dma_start(perm64_sb[:], perm64.ap())
    perm32_sb = pc.tile([ROPE, ROPE], BF16)
    nc.scalar.dma_start(perm32_sb[:], perm32.ap())
    permb = pc.tile([2 * ROPE, ROPE], BF16)
    nc.scalar.dma_start(permb[32:64, :], perm32.ap())
    biasg_sb = pc.tile([P, E], F32); nc.scalar.dma_start(biasg_sb[:], biasg.ap())
    cosq_sb = px.tile([P, TLOC], BF16)
    nc.gpsimd.dma_start(cosq_sb[:], cosq.ap())
    ssinq_sb = px.tile([P, TLOC], BF16)
    nc.gpsimd.dma_start(ssinq_sb[:], ssinq.ap())

    # ---- K-side produced per 512-token chunk: stats -> ckv -> rope ->
    # k_nope -> V_ext -> per-chunk kf assembly, so the slot-major score loop
    # can start as soon as chunk 0's kf columns land. Q projection is
    # interleaved between chunks.
    pkf = attn.enter_context(tc.tile_pool(name="pkf", bufs=1, side="right"))
    kf = pkf.tile([P, H // 2, T], BF16, name="kf")
    kmask_sb = pkf.tile([P, 16 * QB], BF16)
    nc.gpsimd.dma_start(kmask_sb[:], kmask.ap())
    yall = []
    for yb in range(4):
        yall.append(pkf.tile([P, TLOC], BF16, name=f"yall{yb}"))
    pkv = attn.enter_context(tc.tile_pool(name="pkv", bufs=1, side="right"))
    bc1l = px.tile([P, TLOC], F32)
    vext = [None] * NTB

    def emit_chunk(nt):
        sl = slice(nt * 512, (nt + 1) * 512)
        # rmsnorm stats for this chunk (squares on DVE)
        sps = pacc.tile([1, 512], F32, name="ssq1", tag="accA")
        for cb in range(NCB):
            xq = ptmp1.tile([P, 512], BF16, name="xsq", tag="xsq")
            nc.vector.tensor_tensor(xq[:], xt[cb][:, sl], xt[cb][:, sl],
                                    ALU.mult)
            nc.tensor.matmul(sps[:], ones_128x1[:], xq[:],
                             start=(cb == 0), stop=(cb == NCB - 1))
        rr = ptmp1.tile([1, 512], F32, name="rms1", tag="rms1", bufs=1)
        nc.scalar.activation(rr[:], sps[:], AF.Sqrt, bias=eps_sb[:],
                             scale=1.0 / C)
        iv = ptmp1.tile([1, 512], F32, name="iv1", tag="iv1", bufs=1)
        nc.vector.reciprocal(iv[:], rr[:])
        ivb = ptmp1.tile([1, 512], BF16, name="ivb", tag="ivb")
        nc.vector.tensor_scalar(ivb[:], iv[:], 1.0, None, op0=ALU.mult)
        bc1 = ptabt.tile([P, 512], BF16, name="bc1", tag="bc1")
        nc.gpsimd.partition_broadcast(bc1[:], ivb[:])
        # ckv
        lat_ps = pps.tile([P, 512], F32, name="latps", tag="psA")
        rop_ps = pps.tile([ROPE, 512], F32, name="ropps", tag="psA")
        for cb in range(NCB):
            nc.tensor.matmul(lat_ps[:], wkva_sb[cb][:, 0:R], xt[cb][:, sl],
                             start=(cb == 0), stop=(cb == NCB - 1))
        for cb in range(NCB):
            nc.tensor.matmul(rop_ps[:], wkva_sb[cb][:, R:R + ROPE],
                             xt[cb][:, sl],
                             start=(cb == 0), stop=(cb == NCB - 1))
        kvlat = pkv1.tile([R, 512], BF16, name="kvlat", tag="kvl", bufs=2)
        nc.vector.tensor_tensor(kvlat[:], lat_ps[:], bc1[:], ALU.mult)
        kropef = ptabt.tile([ROPE, 512], BF16, name="kropef", tag="krf")
        nc.vector.tensor_tensor(kropef[:], rop_ps[:], bc1[0:ROPE, :],
                                ALU.mult)
        # rope K
        par_ps = pps.tile([ROPE, 512], F32, name="parps", tag="psA")
        nc.tensor.matmul(par_ps[:], perm32_sb[:], kropef[:])
        t1 = ptabt.tile([ROPE, 512], F32, name="kr1", tag="kr1")
        nc.gpsimd.tensor_tensor(t1[:], kropef[:], ktab[0:ROPE, sl], ALU.mult)
        t2 = ptabt.tile([ROPE, 512], F32, name="kr2", tag="kr2")
        nc.vector.tensor_tensor(t2[:], par_ps[:],
                                ktab[ROPE:2 * ROPE, sl], ALU.mult)
        kropebf = pkv1.tile([ROPE, 512], BF16, name="kropebf", tag="krb",
                            bufs=2)
        nc.gpsimd.tensor_tensor(kropebf[:], t1[:], t2[:], ALU.add)
        # k_nope (chunk-local)
        knopea = pkx.tile([P, 4, 512], BF16, name="knopea", tag="kno",
                          bufs=2)
        for mb4 in range(4):
            ps = pps.tile([P, 512], F32, name="knps", tag="psA")
            nc.tensor.matmul(ps[:], wkvb_sb[:, mb4 * P:(mb4 + 1) * P],
                             kvlat[:])
            nc.vector.tensor_scalar(knopea[:, mb4, :], ps[:], 1.0, None,
                                    op0=ALU.mult)
        # V_ext for this chunk's four key blocks
        for j in range(4):
            tb = 4 * nt + j
            tl = pkv.tile([P, H, 33], BF16, name=f"vext{tb}")
            ps = pps.tile([P, H * NOPE], F32, name="vps", tag="psA")
            nc.tensor.matmul(ps[:], kvlat[:, j * P:(j + 1) * P], wkvb_sb[:])
            nc.scalar.copy(tl[:, :, 0:NOPE],
                           ps[:].rearrange("p (h d) -> p h d", h=H))
            nc.any.memset(tl[:, :, NOPE:NOPE + 1], 1.0)
            vext[tb] = tl
        # assemble this chunk's kf columns (rows: [h_even|rope|h_odd|rope])
        nc.sync.dma_start(kf[0:32, 0::2, sl], knopea[0:32, :, :])
        nc.scalar.dma_start(kf[0:32, 1::2, sl], knopea[64:96, :, :])
        nc.sync.dma_start(kf[64:96, 0::2, sl], knopea[32:64, :, :])
        nc.scalar.dma_start(kf[64:96, 1::2, sl], knopea[96:128, :, :])
        kropeb = kropebf[:].rearrange("p (o t) -> p o t", o=1).broadcast_to(
            (ROPE, H // 2, 512))
        nc.sync.dma_start(kf[32:64, :, sl], kropeb)
        nc.scalar.dma_start(kf[96:128, :, sl], kropeb)

    def emit_local_stats():
        spsl = pacc.tile([1, TLOC], F32, name="ssql", tag="accA")
        for cb in range(NCB):
            xq = ptmp1.tile([P, TLOC], BF16, name="xsql", tag="xsq")
            nc.vector.tensor_tensor(xq[:], xlbf[cb][:], xlbf[cb][:],
                                    ALU.mult)
            nc.tensor.matmul(spsl[:], ones_128x1[:], xq[:],
                             start=(cb == 0), stop=(cb == NCB - 1))
        rrl = ptmp1.tile([1, TLOC], F32, name="rmsl", tag="rms1", bufs=1)
        nc.scalar.activation(rrl[:], spsl[:], AF.Sqrt, bias=eps_sb[:],
                             scale=1.0 / C)
        invr1l = ptmp1.tile([1, TLOC], F32, name="iv1l", tag="iv1l", bufs=1)
        nc.vector.reciprocal(invr1l[:], rrl[:])
        nc.gpsimd.partition_broadcast(bc1l[:], invr1l[:])
        for cb in range(NCB):
            nc.vector.tensor_tensor(xlbf[cb][:], xlbf[cb][:], bc1l[:],
                                    ALU.mult)

    def qproj(mb):
        wqm = px.tile([P, NCB, P], BF16, name="wqm", tag="wqm", bufs=3)
        nc.gpsimd.dma_start(wqm[:], wq.ap()[:, mb])
        tl = pkf.tile([P, TLOC], BF16, name=f"qbf{mb}")
        ps = pps.tile([P, TLOC], F32, name="qps", tag="psA")
        for cb in range(NCB):
            nc.tensor.matmul(ps[:], wqm[:, cb, :],
                             xlbf[cb][:],
                             start=(cb == 0), stop=(cb == NCB - 1))
        nc.scalar.copy(tl[:], ps[:])
        # rope reads tl's scaled bf16 rope rows in place; only the base-96
        # head needs a staging copy (PE/gpsimd base limits)
        qr2 = pq_t.tile([ROPE, TLOC], BF16, name="qr2", tag="qr2", bufs=1)
        nc.vector.tensor_scalar(qr2[:], tl[96:128, :], 1.0, None,
                                op0=ALU.mult)
        par = pps.tile([2 * ROPE, TLOC], F32, name="qpar", tag="psA")
        nc.tensor.matmul(par[0:ROPE, :], permb[32:64, :], tl[32:64, :])
        nc.tensor.matmul(par[ROPE:2 * ROPE, :], perm32_sb[:], qr2[:])
        t1 = pq_t.tile([2 * ROPE, TLOC], BF16, name="qt1", tag="qt1")
        nc.gpsimd.tensor_tensor(t1[0:ROPE, :], tl[32:64, :],
                                cosq_sb[32:64, :], ALU.mult)
        nc.gpsimd.tensor_tensor(t1[ROPE:2 * ROPE, :], tl[96:128, :],
                                cosq_sb[96:128, :], ALU.mult)
        t2 = pq_t.tile([2 * ROPE, TLOC], BF16, name="qt2", tag="qt2")
        nc.vector.tensor_tensor(t2[:], par[:], ssinq_sb[0:2 * ROPE, :],
                                ALU.mult)
        nc.vector.tensor_tensor(tl[32:64, :], t1[0:ROPE, :],
                                t2[0:ROPE, :], ALU.add)
        nc.vector.tensor_tensor(tl[96:128, :], t1[ROPE:2 * ROPE, :],
                                t2[ROPE:2 * ROPE, :], ALU.add)
        return tl

    emit_chunk(0)
    emit_local_stats()
    qall = [qproj(0), qproj(1)]
    emit_chunk(1)
    qall += [qproj(2), qproj(3)]
    emit_chunk(2)
    qall += [qproj(4), qproj(5)]
    emit_chunk(3)
    qall += [qproj(6), qproj(7)]

    xload.close()
    tabs.close()
    pkxst.close()
    kvst.close()
    early.close()

    # stage the first MoE experts now: this pool reuses the just-freed
    # ramp-pool space, so these loads overlap the score phase instead of
    # waiting for kf/vext to die
    NPRE = 6
    late = contextlib.ExitStack()
    pw1 = late.enter_context(tc.tile_pool(name="wpre", bufs=1))
    wts = []
    for ei in range(NPRE):
        g8 = pw1.tile([P, NCB // 2, 2, I], F8, name=f"w8g{ei}")
        nc.sync.dma_start(g8[:], w8g.ap()[ei])
        u8 = pw1.tile([P, NCB // 2, 2, I], F8, name=f"w8u{ei}")
        nc.sync.dma_start(u8[:], w8u.ap()[ei])
        d8 = pw1.tile([P, NIB // 2, 2, C], F8, name=f"w8d{ei}")
        nc.gpsimd.dma_start(d8[:], w8d.ap()[ei])
        wts.append((g8, u8, d8))

    # deferred loads for the Wo phase (issue during scores)
    pq2  = attn.enter_context(tc.tile_pool(name="pq2", bufs=1))
    xloc = []
    for cb in range(NCB):
        tl = pq2.tile([P, TLOC], F32, name="xloc", tag="xloc",
                      bufs=5)
        nc.sync.dma_start(tl[:], xlocT.ap()[cb * P:(cb + 1) * P, :])
        xloc.append(tl)
    wo_sb = []
    for kb in range(4):
        tl = pq2.tile([P, C], BF16, name=f"wos{kb}")
        nc.sync.dma_start(tl[:], wo.ap()[kb * P:(kb + 1) * P, :])
        wo_sb.append(tl)
    wgate_sb = []
    for cb in range(NCB):
        tl = pq2.tile([P, E], F32, name=f"wgate{cb}")
        nc.sync.dma_start(tl[:], wgate.ap()[cb * P:(cb + 1) * P, :])
        wgate_sb.append(tl)
    # ---- slot-major score loop: slot sl4 only needs kf chunks [0, 4-sl4),
    # so slot 3 starts right after chunk 0 is assembled. Per (slot, head)
    # the y accumulation is a quarter-bank [33, QB] PSUM tile.
    ptmp2 = attn.enter_context(tc.tile_pool(name="tmp2", bufs=2, side="right"))
    pE   = attn.enter_context(tc.tile_pool(name="pE", bufs=4, side="right"))
    for sl4 in (3, 2, 1, 0):
        nkb = KB_SLOT[sl4]
        ngr = nkb // 4
        csl = slice(sl4 * QB, (sl4 + 1) * QB)
        for h in range(H):
            mb, po = h // 2, (h % 2) * 64
            tl = qall[mb]
            y_ps = pacc.tile([NOPE + 1, QB], F32, name="yps", tag="accA")
            # key-block groups in pairs: an 8-block PSUM tile and a single
            # 1024-wide exp amortize the Act access penalty
            pairs = [(0, min(2, ngr))] + ([(2, ngr)] if ngr > 2 else [])
            for (p0, p1) in pairs:
                ng = p1 - p0
                s8 = pps.tile([P, 4 * ng, QB], F32, name="sps",
                              tag=("s8" if ng == 2 else "psA"), bufs=2)
                for i in range(4 * ng):
                    kb = 4 * p0 + i
                    nc.tensor.matmul(s8[:, i, :],
                                     kf[po:po + 64, mb, kb * P:(kb + 1) * P],
                                     tl[po:po + 64, csl])
                ee = pE.tile([P, 4 * ng * QB], BF16, name="ee", tag="ee",
                             bufs=3)
                nc.scalar.activation(
                    ee[:], s8[:].rearrange("a b c -> a (b c)"),
                    AF.Exp, scale=0.125)
                # only the last group holds the causal boundary/future blocks
                nlast = 4 * (ngr - 1 - p0)
                if p1 == ngr:
                    mcol = sl4 * 4 * QB
                    em = pE.tile([P, 4 * QB], BF16, name="em", tag="em",
                                 bufs=2)
                    nc.vector.tensor_tensor(
                        em[:], ee[:, nlast * QB:(nlast + 4) * QB],
                        kmask_sb[:, mcol:mcol + 4 * QB], ALU.mult)
                    emv = em[:].rearrange("a (b c) -> a b c", b=4)
                eev = ee[:].rearrange("a (b c) -> a b c", b=4 * ng)
                for i in range(4 * ng):
                    kb = 4 * p0 + i
                    lastg = (p1 == ngr and i >= nlast)
                    srcv = emv[:, i - nlast, :] if lastg else eev[:, i, :]
                    nc.tensor.matmul(y_ps[:], vext[kb][:, h, 0:NOPE + 1],
                                     srcv,
                                     start=(p0 == 0 and i == 0),
                                     stop=(p1 == ngr and i == 4 * ng - 1),
                                     skip_group_check=True)
            rr = ptmp2.tile([1, QB], F32, name="rr", tag="rr")
            nc.vector.reciprocal(rr[:], y_ps[NOPE:NOPE + 1, :])
            rb = ptmp2.tile([NOPE, QB], F32, name="rb", tag="rb")
            nc.gpsimd.partition_broadcast(rb[:], rr[:])
            yt = yall[h // 4]
            ro = (h % 4) * NOPE
            nc.vector.tensor_tensor(yt[ro:ro + NOPE, csl],
                                    y_ps[0:NOPE, :], rb[:], ALU.mult)

    # ---- Wo + residual -> xa^T (kept in f32 to the end); cb-outer so each
    # xa block finishes early and rmsnorm2 squares overlap the remaining Wo
    xa = []
    for cb in range(NCB):
        xa.append(pmx.tile([P, TLOC], F32, name=f"xa{cb}"))
    sps2 = pacc.tile([1, TLOC], F32, name="ssq2", tag="accA")
    for cb in range(NCB):
        ps = pps.tile([P, TLOC], F32, name="ops", tag="psA")
        for kb in range(4):
            nc.tensor.matmul(ps[:], wo_sb[kb][:, cb * P:(cb + 1) * P],
                             yall[kb][:],
                             start=(kb == 0), stop=(kb == 3))
        nc.vector.scalar_tensor_tensor(xa[cb][:], ps[:], 1.0,
                                       xloc[cb][:],
                                       op0=ALU.mult, op1=ALU.add)
        xq = ptmp2.tile([P, TLOC], BF16, name="xsq2", tag="xsqB", bufs=1)
        nc.vector.tensor_tensor(xq[:], xa[cb][:], xa[cb][:], ALU.mult)
        nc.tensor.matmul(sps2[:], ones_128x1[:], xq[:],
                         start=(cb == 0), stop=(cb == NCB - 1))
    if DEBUG:
        for cb in range(NCB):
            nc.sync.dma_start(dbg["d_xaT"].ap()[cb * P:(cb + 1) * P, :],
                              xa[cb][:])

    # ---- rmsnorm2 + xmoe (MoE-phase pool pmx)
    invr2 = pmx.tile([1, TLOC], F32)
    rr2 = ptmp2.tile([1, TLOC], F32, name="rms2", tag="rmsB", bufs=1)
    nc.scalar.activation(rr2[:], sps2[:], AF.Sqrt, bias=eps_sb[:], scale=1.0 / C)
    nc.vector.reciprocal(invr2[:], rr2[:])
    if DEBUG:
        nc.sync.dma_start(dbg["d_invr2"].ap(), invr2[:])
    bc2 = pmx.tile([P, TLOC], F32)
    nc.gpsimd.partition_broadcast(bc2[:], invr2[:])
    xmoe8 = pmx.tile([P, NCB, TLOC], F8, name="xmoe8")
    for cb in range(NCB):
        nc.vector.tensor_tensor(xmoe8[:, cb, :], xa[cb][:], bc2[:], ALU.mult)
    if DEBUG:
        for cb in range(NCB):
            xmc = ptmp2.tile([P, TLOC], F32, name="xmc", tag="xmc")
            nc.vector.tensor_scalar(xmc[:], xmoe8[:, cb, :], 1.0, None,
                                    op0=ALU.mult)
            nc.sync.dma_start(dbg["d_xmoe"].ap()[cb * P:(cb + 1) * P, :],
                              xmc[:])

    # ---- gate (fp32); gating weights reach partitions via row-broadcasts
    bcomb = [pmx.tile([P, TLOC], BF16, name=f"bcomb{e}") for e in range(E)]
    ctflat = pmx.tile([1, 4 * E * P], BF16, name="ctflat")
    cmfall = pmx.tile([P, 4, E], BF16, name="cmfall")
    for tb in range(4):
        tsl = slice(tb * P, (tb + 1) * P)
        g_ps = pps.tile([P, E], F32, name="gps", tag="psA")
        for cb in range(NCB):
            nc.tensor.matmul(g_ps[:], xa[cb][:, tsl], wgate_sb[cb][:],
                             start=(cb == 0), stop=(cb == NCB - 1))
        ir_ps = pps.tile([P, 1], F32, name="irps", tag="psA")
        nc.tensor.transpose(ir_ps[:], invr2[:, tsl], ones1f[:])
        ir_col = ptmp2.tile([P, 1], F32, name="ircol", tag="ircol")
        nc.scalar.copy(ir_col[:], ir_ps[:])
        lg = ptmp2.tile([P, E], F32, name="lg", tag="lg")
        nc.vector.scalar_tensor_tensor(lg[:], g_ps[:], ir_col[:], biasg_sb[:],
                                       op0=ALU.mult, op1=ALU.add)
        m1 = ptmp2.tile([P, 1], F32, name="m1", tag="m1")
        nc.vector.reduce_max(m1[:], lg[:], axis=mybir.AxisListType.X)
        eq1 = ptmp2.tile([P, E], F32, name="eq1", tag="eq1")
        nc.vector.tensor_scalar(eq1[:], lg[:], m1[:], None, op0=ALU.is_equal)
        lm = ptmp2.tile([P, E], F32, name="lm", tag="lm")
        nc.vector.scalar_tensor_tensor(lm[:], eq1[:], -1e9, lg[:],
                                       op0=ALU.mult, op1=ALU.add)
        m2 = ptmp2.tile([P, 1], F32, name="m2", tag="m2")
        nc.vector.reduce_max(m2[:], lm[:], axis=mybir.AxisListType.X)
        eq2 = ptmp2.tile([P, E], F32, name="eq2", tag="eq2")
        nc.vector.tensor_scalar(eq2[:], lm[:], m2[:], None, op0=ALU.is_equal)
        dm = ptmp2.tile([P, 1], F32, name="dm", tag="dm")
        nc.vector.tensor_scalar(dm[:], m1[:], m2[:], None, op0=ALU.subtract)
        w1 = ptmp2.tile([P, 1], F32, name="w1", tag="w1")
        nc.scalar.activation(w1[:], dm[:], AF.Sigmoid)
        w2 = ptmp2.tile([P, 1], F32, name="w2", tag="w2")
        nc.vector.tensor_scalar(w2[:], w1[:], -1.0, 1.0, op0=ALU.mult,
                                op1=ALU.add)
        cmb = ptmp2.tile([P, E], F32, name="cmb", tag="cmb")
        nc.vector.tensor_scalar(cmb[:], eq1[:], w1[:], None, op0=ALU.mult)
        cm2 = ptmp2.tile([P, E], F32, name="cm2", tag="cm2")
        nc.vector.tensor_scalar(cm2[:], eq2[:], w2[:], None, op0=ALU.mult)
        nc.vector.tensor_tensor(cmfall[:, tb, :], cmb[:], cm2[:], ALU.add)
        if DEBUG:
            nc.sync.dma_start(dbg["d_comb"].ap()[:, tb * E:(tb + 1) * E],
                              cmfall[:, tb, :])


    # flatten (tokens-on-partitions, [tb,e] free) onto one partition row via
    # DMA, then broadcast each expert's row with a strided source AP
    nc.sync.dma_start(ctflat[:], cmfall[:])
    ctv = ctflat[0:1, :].rearrange("o (p t e) -> o t p e", p=P, t=4)
    for e in range(E):
        nc.gpsimd.partition_broadcast(bcomb[e][:], ctv[:, :, :, e])

    attn.close()

    # ---- MoE: fp8 DoubleRow matmuls, all 9 experts' weights resident,
    # down-projection accumulated across experts in 4 PSUM banks per chunk.
    moe = contextlib.ExitStack()
    pw   = moe.enter_context(tc.tile_pool(name="wmoe", bufs=1))
    pgu  = moe.enter_context(tc.tile_pool(name="psG", bufs=4, space="PSUM"))
    pwd  = moe.enter_context(tc.tile_pool(name="psD", bufs=1, space="PSUM"))
    pmoe = moe.enter_context(tc.tile_pool(name="hmoe", bufs=5))
    php  = moe.enter_context(tc.tile_pool(name="hhp", bufs=2 if DEBUG else 3))

    for ei in range(NPRE, E + 1):
        g8 = pw.tile([P, NCB // 2, 2, I], F8, name=f"w8g{ei}")
        nc.sync.dma_start(g8[:], w8g.ap()[ei])
        u8 = pw.tile([P, NCB // 2, 2, I], F8, name=f"w8u{ei}")
        nc.sync.dma_start(u8[:], w8u.ap()[ei])
        d8 = pw.tile([P, NIB // 2, 2, C], F8, name=f"w8d{ei}")
        nc.gpsimd.dma_start(d8[:], w8d.ap()[ei])
        wts.append((g8, u8, d8))

    fo = [pmoe.tile([P, TLOC], F32, name=f"fo{cb}", bufs=1)
          for cb in range(NCB)]
    for ch in range(2):
        csl = slice(ch * CHUNK, (ch + 1) * CHUNK)
        acc = [pwd.tile([P, 2 * CHUNK], F32, name=f"dacc{j}", tag=f"dacc{j}")
               for j in range(NCB // 2)]
        order = list(range(E + 1)) if ch == 0 else list(range(E, -1, -1))

        def emit_gu(ei):
            g8, u8, d8 = wts[ei]
            bc = None if ei == 0 else bcomb[ei - 1]
            hh8 = php.tile([P, NIB, CHUNK], F8, name="hh8", tag="hh8")
            for jp in range(2):              # I-block pairs
                gp = pgu.tile([P, 2 * CHUNK], F32, name="gp", tag="psG")
                up = pgu.tile([P, 2 * CHUNK], F32, name="up", tag="psG")
                for i2 in range(2):
                    ib = 2 * jp + i2
                    isl = slice(ib * P, (ib + 1) * P)
                    osl = slice(i2 * CHUNK, (i2 + 1) * CHUNK)
                    for j in range(4):
                        nc.tensor.matmul(gp[:, osl], g8[:, j, :, isl],
                                         xmoe8[:, 2 * j:2 * j + 2, csl],
                                         perf_mode=DR,
                                         start=(j == 0), stop=(j == 3))
                    for j in range(4):
                        nc.tensor.matmul(up[:, osl], u8[:, j, :, isl],
                                         xmoe8[:, 2 * j:2 * j + 2, csl],
                                         perf_mode=DR,
                                         start=(j == 0), stop=(j == 3))
                silu = pmoe.tile([P, 2 * CHUNK], BF16, name="silu", tag="silu")
                nc.scalar.activation(silu[:], gp[:], AF.Silu, scale=1.0 / WS)
                if bc is not None:
                    sgc = pmoe.tile([P, 2 * CHUNK], BF16, name="sgc", tag="sgc")
                    for i2 in range(2):
                        osl = slice(i2 * CHUNK, (i2 + 1) * CHUNK)
                        nc.vector.tensor_tensor(sgc[:, osl], silu[:, osl],
                                                bc[:, csl], ALU.mult)
                else:
                    sgc = silu
                # hh8 = 4*h (up carries 2^6; 2^-4 here keeps fp8 in range)
                nc.vector.scalar_tensor_tensor(
                    hh8[:, 2 * jp:2 * jp + 2, :], up[:], 4.0 / WS, sgc[:],
                    op0=ALU.mult, op1=ALU.mult)
            return hh8

        def emit_dacc(n, ei, hh8):
            d8 = wts[ei][2]
            first, last = (n == 0), (n == E)
            for j in range(NCB // 2):
                for c2 in range(2):
                    cb = 2 * j + c2
                    osl = slice(c2 * CHUNK, (c2 + 1) * CHUNK)
                    for jp in range(2):
                        # the pass's first matmul per bank zeroes the whole
                        # bank (both c2 halves) via start_tensor_calc
                        nc.tensor.matmul(
                            acc[j][:, osl], d8[:, jp, :, cb * P:(cb + 1) * P],
                            hh8[:, 2 * jp:2 * jp + 2, :], perf_mode=DR,
                            start=(first and c2 == 0 and jp == 0),
                            stop=(last and jp == 1),
                            skip_group_check=True)

        # software-pipelined: expert e+1's gate/up matmuls are emitted before
        # expert e's down accumulation so PE never waits on the silu/hh8 chain
        def flush_bank(j):
            for c2 in range(2):
                cb = 2 * j + c2
                osl = slice(c2 * CHUNK, (c2 + 1) * CHUNK)
                nc.vector.scalar_tensor_tensor(
                    fo[cb][:, csl], acc[j][:, osl], 1.0 / (WS * 4.0),
                    xa[cb][:, csl], op0=ALU.mult, op1=ALU.add)
                eng = (nc.sync, nc.scalar, nc.gpsimd)[cb % 3]
                eng.dma_start(outT.ap()[cb * P:(cb + 1) * P, csl],
                              fo[cb][:, csl])

        def emit_dacc_final(n, ei, hh8):
            # bank-major so each acc bank flushes while the next is finishing
            d8 = wts[ei][2]
            for j in range(NCB // 2):
                for c2 in range(2):
                    cb = 2 * j + c2
                    osl = slice(c2 * CHUNK, (c2 + 1) * CHUNK)
                    for jp in range(2):
                        nc.tensor.matmul(
                            acc[j][:, osl], d8[:, jp, :, cb * P:(cb + 1) * P],
                            hh8[:, 2 * jp:2 * jp + 2, :], perf_mode=DR,
                            start=False, stop=(jp == 1),
                            skip_group_check=True)
                flush_bank(j)

        pending = []
        for n, ei in enumerate(order):
            hh8 = emit_gu(ei)
            pending.append((n, ei, hh8))
            if len(pending) > 2:
                emit_dacc(*pending.pop(0))
        while pending:
            args = pending.pop(0)
            if pending:
                emit_dacc(*args)
            else:
                emit_dacc_final(*args)



    moe.close()
    late.close()
    whole.close()


# =============================================================== host side
def _build():
    if "nc" in _CACHE:
        return _CACHE["nc"]
    nc = bacc.Bacc("TRN2", target_bir_lowering=False, debug=False,
                   num_devices=8)
    with tile.TileContext(nc) as tc:
        _emit(nc, tc)
    nc.compile()
    _CACHE["nc"] = nc
    return nc


def _rope_tables(pos):
    # pos: (N,) positions; returns cos,ssin of shape (ROPE, N) in the
    # row-pair layout (rows 2i/2i+1 both carry angle pos*freq_i; ssin row 2i
    # is -sin, row 2i+1 is +sin).
    freqs = 1.0 / (THETA ** (np.arange(0, ROPE, 2, dtype=np.float32) / ROPE))
    ang = np.outer(freqs, pos.astype(np.float32))          # (16, N)
    cos = np.repeat(np.cos(ang), 2, axis=0).astype(np.float32)
    sin = np.sin(ang).astype(np.float32)
    ssin = np.empty((ROPE, len(pos)), np.float32)
    ssin[0::2] = -sin
    ssin[1::2] = sin
    return cos, ssin


def _host_inputs(inputs, core):
    bf = lambda a: np.ascontiguousarray(a).astype(ml_dtypes.bfloat16)
    f32 = lambda a: np.ascontiguousarray(a, dtype=np.float32)
    b, q = core // 4, core % 4
    gq = [15 - q, 11 - q, 7 - q, 3 - q]   # query block (of 128) per slot
    x = np.asarray(inputs["x"], np.float32)
    w_ln1 = np.asarray(inputs["w_ln1"], np.float32)
    w_ln2 = np.asarray(inputs["w_ln2"], np.float32)
    xT = x[b].T                                            # (C, T)
    loc_cols = np.concatenate([np.arange(g * QB, (g + 1) * QB) for g in gq])
    xloc = xT[:, loc_cols]

    # rope tables
    posq = loc_cols.astype(np.float32)
    cq, sq = _rope_tables(posq)
    cosq = np.zeros((P, TLOC), np.float32)                 # rows match tl
    cosq[32:64] = cq
    cosq[96:128] = cq
    ssinq = np.zeros((P, TLOC), np.float32)
    ssinq[0:32] = sq
    ssinq[32:64] = sq
    ssinq[96:128] = sq
    posk = np.arange(T, dtype=np.float32)
    cosk, ssink = _rope_tables(posk)

    # permutation matrices (pair swap)
    p32 = np.zeros((ROPE, ROPE), np.float32)
    for i in range(ROPE // 2):
        p32[2 * i + 1, 2 * i] = 1.0
        p32[2 * i, 2 * i + 1] = 1.0
    p64 = np.zeros((2 * ROPE, 2 * ROPE), np.float32)
    p64[:ROPE, :ROPE] = p32
    p64[ROPE:, ROPE:] = p32

    # causal masks for each slot's LAST 4-kb group only (the causal boundary
    # and all fully-future blocks fall there since q <= 3)
    kmask = np.zeros((P, 16 * QB), np.float32)
    ki = np.arange(P)[:, None]
    qi = np.arange(QB)[None, :]
    for sl4 in range(4):
        g = gq[sl4]
        base = sl4 * 4 * QB
        for j, kb in enumerate(range(KB_SLOT[sl4] - 4, KB_SLOT[sl4])):
            m = np.zeros((P, QB), np.float32)
            if kb < g:
                m[:] = 1.0
            elif kb == g:
                m = (ki <= qi).astype(np.float32)
            kmask[:, base + j * QB: base + (j + 1) * QB] = m

    wq0 = np.asarray(inputs["Wq"], np.float32) * w_ln1[:, None]
    # packed per-mb layout: wqp[p, mb, cb, c] = wq[cb*128+p, mb*128+c]
    wq = np.ascontiguousarray(
        wq0.reshape(NCB, P, NCB, P).transpose(1, 2, 0, 3))
    wkva = np.asarray(inputs["Wkva"], np.float32) * w_ln1[:, None]
    wo_nope = np.asarray(inputs["Wo"], np.float32).reshape(H, D, C)[:, :NOPE]
    wgate = np.asarray(inputs["Wgate"], np.float32) * w_ln2[:, None]
    biasg = np.broadcast_to(np.asarray(inputs["expert_bias"], np.float32),
                            (P, E)).copy()

    # fp8 expert weights: x2^6 pre-scale, DoubleRow k-tile packing.
    f8 = ml_dtypes.float8_e4m3
    wg_all = np.concatenate([np.asarray(inputs["sWg"], np.float32),
                             np.asarray(inputs["Wg"], np.float32)], axis=0)
    wu_all = np.concatenate([np.asarray(inputs["sWu"], np.float32),
                             np.asarray(inputs["Wu"], np.float32)], axis=0)
    wd_all = np.concatenate([np.asarray(inputs["sWd"], np.float32),
                             np.asarray(inputs["Wd"], np.float32)], axis=0)
    wg_all = wg_all * w_ln2[None, :, None] * WS
    wu_all = wu_all * w_ln2[None, :, None] * WS
    wd_all = wd_all * WS
    # (E+1, C, I) -> (E+1, 128, C/256, 2, I);  C index = (2j+kt)*128 + c
    w8g = np.ascontiguousarray(
        wg_all.reshape(E + 1, NCB // 2, 2, P, I)
        .transpose(0, 3, 1, 2, 4)).astype(f8)
    w8u = np.ascontiguousarray(
        wu_all.reshape(E + 1, NCB // 2, 2, P, I)
        .transpose(0, 3, 1, 2, 4)).astype(f8)
    # (E+1, I, C) -> (E+1, 128, I/256, 2, C)
    w8d = np.ascontiguousarray(
        wd_all.reshape(E + 1, NIB // 2, 2, P, C)
        .transpose(0, 3, 1, 2, 4)).astype(f8)

    f8c = lambda a: np.ascontiguousarray(a).astype(ml_dtypes.float8_e4m3)
    m = {
        "xT_f8": f8c(xT),
        "xlocT": f32(xloc),
        "xlocbf": bf(xloc),
        "wq": bf(wq),
        "wkva": f8c(wkva),
        "wkvb": bf(inputs["Wkvb"]),
        "wo": bf(wo_nope.reshape(H * NOPE, C)),
        "cosq": bf(cosq), "ssinq": bf(ssinq),
        "cosk": bf(cosk), "ssink": bf(ssink),
        "perm64": bf(p64), "perm32": bf(p32),
        "ident": np.eye(P, dtype=np.float32),
        "kmask": bf(kmask),
        "wgate": f32(wgate),
        "biasg": biasg,
        "w8g": w8g,
        "w8u": w8u,
        "w8d": w8d,
    }
    return m


def kernel(**inputs):
    nc = _build()
    in_maps = [_host_inputs(inputs, core) for core in range(8)]
    res = bass_utils.run_bass_kernel_spmd(nc, in_maps, core_ids=list(range(8)))
    out = np.empty((B, T, C), np.float32)
    for core in range(8):
        b, q = core // 4, core % 4
        oT = res.results[core]["outT"]                     # (C, 512)
        for sl4, g in enumerate([15 - q, 11 - q, 7 - q, 3 - q]):
            out[b, g * QB:(g + 1) * QB] = oT[:, sl4 * QB:(sl4 + 1) * QB].T
    return out

